# revision 1
# baseline (speedup 1.0000x reference)
"""BERT self-attention (BS=4, SEQ=2048, HID=768, NH=12) on 8 NeuronCores.

Sharding: core c -> batch b = c//2, head-group g = c%2 (6 heads each).
Per core the Bass kernel computes, for its batch element and 6 heads:
  Q^T/K^T = (Wh @ X^T + b)   in [d, q] layout  (d on partitions)
  V       = X @ Wv^T + bv    in [k, d] layout, rows scaled by mask m[k]
  S^T     = K^T.T-free matmul -> [k_block, q] scores in PSUM
  P^T     = exp(S^T / 8)     (ACT, PSUM->SBUF; mask folded into V)
  ctx^T   = V'.T @ P^T accumulated over k blocks, with a 65th row = mask
            column giving the softmax denominator.
  out     = ctx^T[0:64] * broadcast(1/denom)  -> [64, q] per head
Host does input transposes (free), sharding, and the final [d,q]->[q,d]
untranspose + concat.

Biases are folded in via an appended ones-row on X^T (contraction 769).
"""

import numpy as np

import concourse.bass as bass
import concourse.tile as tile
from concourse import bacc
from concourse import mybir
from concourse.bass_utils import run_bass_kernel_spmd

F32 = mybir.dt.float32
F32R = mybir.dt.float32r
F16 = mybir.dt.float16
DT_MM = F16          # dtype for matmul operands (A/B: F16 vs F32R)
DT_NP = np.float16   # matching numpy dtype for host-side input prep

BS, SEQ, HID, NH, HD = 4, 2048, 768, 12, 64
NCORES = 8
HPC = 6          # heads per core
FCH = 6          # 128-row chunks of the 768 contraction dim
DSH = HPC * HD   # 384 output features per core


def _body(tc, xt_d, wq_d, wk_d, wv_d, mt_d, ot_d):
    nc = tc.nc
    Exp = mybir.ActivationFunctionType.Exp

    with tc.tile_pool(name="persist", bufs=1) as persist:
        # Warm the exp table set ASAP (overlaps the input DMAs).
        dummy = persist.tile([1, 1], F32, tag="dummy")
        nc.vector.memset(dummy, 0.0)
        nc.scalar.activation(out=dummy, in_=dummy, func=Exp)

        mtile = persist.tile([128, 16], DT_MM, tag="mtile")
        nc.sync.dma_start(out=mtile, in_=mt_d[:, :])
        mtf = persist.tile([128, 16], F32, tag="mtf")
        nc.vector.tensor_copy(out=mtf, in_=mtile)
        qt = [persist.tile([128, SEQ], DT_MM, tag=f"qt{j}", name=f"qt{j}") for j in range(3)]
        kt = [persist.tile([128, SEQ], DT_MM, tag=f"kt{j}", name=f"kt{j}") for j in range(3)]
        vt = persist.tile([128, 16, DSH], DT_MM, tag="vt")

        # ---------------- Phase 1: QKV projections ----------------
        with tc.tile_pool(name="xw", bufs=1) as xw, \
             tc.tile_pool(name="qkp", bufs=3, space="PSUM") as qkp, \
             tc.tile_pool(name="vp", bufs=2, space="PSUM") as vp:
            xts = []
            for f in range(FCH):
                t = xw.tile([128, SEQ], DT_MM, tag=f"x{f}")
                nc.sync.dma_start(out=t, in_=xt_d[f * 128:(f + 1) * 128, :])
                xts.append(t)
            xt1 = persist.tile([1, SEQ], DT_MM, tag="x6")
            nc.sync.dma_start(out=xt1, in_=xt_d[768:769, :])

            wmap = {}
            for dram, nm in ((wq_d, "q"), (wk_d, "k"), (wv_d, "v")):
                lst = []
                for f in range(FCH):
                    t = xw.tile([128, DSH], DT_MM, tag=f"w{nm}{f}")
                    nc.sync.dma_start(out=t, in_=dram[f * 128:(f + 1) * 128, :])
                    lst.append(t)
                b = xw.tile([1, DSH], DT_MM, tag=f"w{nm}b")
                nc.sync.dma_start(out=b, in_=dram[768:769, :])
                lst.append(b)
                wmap[nm] = lst

            # Q^T, K^T: [384, 2048] as 3 tiles of [128, 2048]
            for nm, dst in (("q", qt), ("k", kt)):
                wt = wmap[nm]
                for j in range(3):
                    js = slice(j * 128, (j + 1) * 128)
                    for qc in range(4):
                        qs = slice(qc * 512, (qc + 1) * 512)
                        ps = qkp.tile([128, 512], F32, tag="qk")
                        for f in range(FCH):
                            nc.tensor.matmul(ps, lhsT=wt[f][:, js],
                                             rhs=xts[f][:, qs],
                                             start=(f == 0), stop=False)
                        nc.tensor.matmul(ps, lhsT=wt[6][:, js],
                                         rhs=xt1[:, qs],
                                         start=False, stop=True)
                        nc.scalar.copy(out=dst[j][:, qs], in_=ps)

            # V: [2048, 384] as 16 k-blocks, mask-scaled rows
            wt = wmap["v"]
            for kb in range(16):
                ks = slice(kb * 128, (kb + 1) * 128)
                ps = vp.tile([128, DSH], F32, tag="v")
                for f in range(FCH):
                    nc.tensor.matmul(ps, lhsT=xts[f][:, ks], rhs=wt[f],
                                     start=(f == 0), stop=False)
                nc.tensor.matmul(ps, lhsT=xt1[:, ks], rhs=wt[6],
                                 start=False, stop=True)
                nc.vector.tensor_scalar_mul(
                    out=vt[:, kb, :], in0=ps,
                    scalar1=mtf[:, kb:kb + 1])

        # ---------------- Phase 2: attention ----------------
        # PSUM: scores 2x[128,1024] (4 banks) + ctx pair [128,1024] (2) +
        # denoms [97,1024] (2) = 8 banks.
        with tc.tile_pool(name="sp", bufs=2, space="PSUM") as sp, \
             tc.tile_pool(name="cp", bufs=1, space="PSUM") as cp, \
             tc.tile_pool(name="dp", bufs=1, space="PSUM") as dp, \
             tc.tile_pool(name="pp", bufs=3) as pp, \
             tc.tile_pool(name="ctp", bufs=4) as ctp, \
             tc.tile_pool(name="rdp", bufs=4) as rdp, \
             tc.tile_pool(name="osp", bufs=3) as osp:
            for j in range(3):
                heads = (2 * j, 2 * j + 1)
                ostage = {h: osp.tile([64, SEQ], F32, tag="os", name=f"os{h}")
                          for h in heads}
                for qh in range(2):
                    q0 = qh * 1024
                    cab = cp.tile([128, 1024], F32, tag="c", name="cab")
                    dn = dp.tile([97, 1024], F32, tag="d", name="dn")
                    for kb in range(16):
                        ks = slice(kb * 128, (kb + 1) * 128)
                        sab = [sp.tile([128, 1024], F32, tag="s", name="sab")
                               for _ in range(2)]
                        # scores: 2-head row-packed pairs (K=64 at rows 0/64)
                        for qq in range(2):
                            qs = slice(q0 + qq * 512, q0 + (qq + 1) * 512)
                            osl = slice(qq * 512, (qq + 1) * 512)
                            for i in range(2):
                                rows = slice(64 * i, 64 * (i + 1))
                                nc.tensor.matmul(sab[i][:, osl],
                                                 lhsT=kt[j][rows, ks],
                                                 rhs=qt[j][rows, qs],
                                                 start=True, stop=True)
                        pab = []
                        for i in range(2):
                            p = pp.tile([128, 1024], DT_MM, tag="p", name="ptile")
                            nc.scalar.activation(out=p, in_=sab[i], func=Exp,
                                                 scale=0.125)
                            pab.append(p)
                        st, sp_ = (kb == 0), (kb == 15)
                        # ctx: col-packed pair (head A -> out rows 0-63,
                        # head B -> rows 64-127 of the same PSUM tile)
                        for qq in range(2):
                            osl = slice(qq * 512, (qq + 1) * 512)
                            for i in range(2):
                                nc.tensor.matmul(
                                    cab[64 * i:64 * (i + 1), osl],
                                    lhsT=vt[:, kb, heads[i] * 64:(heads[i] + 1) * 64],
                                    rhs=pab[i][:, osl], start=st, stop=sp_,
                                    skip_group_check=True)
                        # denominators: 4-way col-packed m=1 matmuls
                        # rows 0/32 = heads A/B cols 0:512; 64/96 = cols 512:1024
                        for idx, (i, qq) in enumerate(((0, 0), (1, 0), (0, 1), (1, 1))):
                            osl = slice(qq * 512, (qq + 1) * 512)
                            r = 32 * idx
                            nc.tensor.matmul(dn[r:r + 1, osl],
                                             lhsT=mtile[:, kb:kb + 1],
                                             rhs=pab[i][:, osl],
                                             start=st, stop=sp_,
                                             tile_position=(0, r),
                                             skip_group_check=True)
                    # drain: out = ctx / denom
                    for i in range(2):
                        h = heads[i]
                        ct = ctp.tile([64, 1024], F32, tag="ct")
                        nc.vector.tensor_copy(out=ct, in_=cab[64 * i:64 * (i + 1), :])
                        rd = rdp.tile([1, 1024], DT_MM, tag="rd")
                        nc.vector.tensor_copy(out=rd[:, 0:512],
                                              in_=dn[32 * i:32 * i + 1, 0:512])
                        nc.vector.tensor_copy(out=rd[:, 512:1024],
                                              in_=dn[64 + 32 * i:64 + 32 * i + 1, 512:1024])
                        bc = sp.tile([64, 1024], F32, tag="s")
                        for qq in range(2):
                            osl = slice(qq * 512, (qq + 1) * 512)
                            nc.tensor.matmul(bc[:, osl], lhsT=xt1[:, 0:64],
                                             rhs=rd[:, osl],
                                             start=True, stop=True)
                        rcp = ctp.tile([64, 1024], F32, tag="rcp")
                        nc.vector.reciprocal(rcp, bc)
                        nc.vector.tensor_mul(out=ostage[h][:, q0:q0 + 1024],
                                             in0=ct, in1=rcp)
                for h in heads:
                    nc.sync.dma_start(out=ot_d[h], in_=ostage[h])


def build_nc():
    nc = bacc.Bacc("TRN2")
    xt_d = nc.declare_dram_parameter("xt", [HID + 1, SEQ], DT_MM, isOutput=False)
    wq_d = nc.declare_dram_parameter("wqT", [HID + 1, DSH], DT_MM, isOutput=False)
    wk_d = nc.declare_dram_parameter("wkT", [HID + 1, DSH], DT_MM, isOutput=False)
    wv_d = nc.declare_dram_parameter("wvT", [HID + 1, DSH], DT_MM, isOutput=False)
    mt_d = nc.declare_dram_parameter("mt", [128, 16], DT_MM, isOutput=False)
    ot_d = nc.declare_dram_parameter("OT", [HPC, HD, SEQ], F32, isOutput=True)
    with tile.TileContext(nc) as tc:
        _body(tc, xt_d, wq_d, wk_d, wv_d, mt_d, ot_d)
    nc.finalize()
    return nc


_NC_CACHE = None


def _get_nc():
    global _NC_CACHE
    if _NC_CACHE is None:
        _NC_CACHE = build_nc()
    return _NC_CACHE


def make_in_maps(hidden_states, attention_mask, Wq, bq, Wk, bk, Wv, bv):
    in_maps = []
    for c in range(NCORES):
        b, g = c // 2, c % 2
        hs = slice(g * DSH, (g + 1) * DSH)
        xt = np.empty((HID + 1, SEQ), DT_NP)
        xt[:HID] = hidden_states[b].T
        xt[HID] = 1.0
        m = (attention_mask[b, 0, 0] > -1).astype(DT_NP)
        mt = np.ascontiguousarray(m.reshape(16, 128).T)

        def aug(W, bias):
            wa = np.empty((HID + 1, DSH), DT_NP)
            wa[:HID] = W[hs, :].T
            wa[HID] = bias[hs]
            return wa

        in_maps.append({
            "xt": np.ascontiguousarray(xt),
            "wqT": aug(Wq, bq),
            "wkT": aug(Wk, bk),
            "wvT": aug(Wv, bv),
            "mt": mt,
        })
    return in_maps


def gather_out(results):
    out = np.empty((BS, SEQ, HID), np.float32)
    for c in range(NCORES):
        b, g = c // 2, c % 2
        ot = results[c]["OT"]  # [6, 64, 2048]
        out[b, :, g * DSH:(g + 1) * DSH] = (
            ot.transpose(2, 0, 1).reshape(SEQ, DSH)
        )
    return out


def kernel(hidden_states, attention_mask, Wq, bq, Wk, bk, Wv, bv):
    nc = _get_nc()
    in_maps = make_in_maps(hidden_states, attention_mask,
                           Wq, bq, Wk, bk, Wv, bv)
    res = run_bass_kernel_spmd(nc, in_maps, core_ids=list(range(NCORES)))
    return gather_out(res.results)



# revision 8
# speedup vs baseline: 1.6995x; 1.6995x over previous
"""BERT self-attention (BS=4, SEQ=2048, HID=768, NH=12) on 8 NeuronCores.

Sharding: core c -> batch b = c//2, head-group g = c%2 (6 heads each).
Per core the Bass kernel computes, for its batch element and 6 heads:
  Q^T/K^T = (Wh @ X^T + b)   in [d, q] layout  (d on partitions)
  V       = X @ Wv^T + bv    in [k, d] layout, rows scaled by mask m[k]
  S^T     = per-head [k_block, q] scores in PSUM (two heads row-tiled,
            concurrent on the PE via disjoint 64-row groups)
  P^T     = exp(S^T / 8)     (one ACT instr per head per k-block; the
            ACT engine is the throughput floor of the kernel)
  ctx^T   = V.T @ P^T accumulated over k blocks ([128,1024] PSUM,
            head A rows 0:64, head B rows 64:128)
  denom   = mask-column m=1 matmuls, 4-way column-tiled (concurrent)
  out     = ctx^T * broadcast(1/denom)  -> [64, q] per head
The drain uses reciprocal_approx_fast (single DVE op, ~5x faster than
reciprocal) on the [2,1024] denominator rows, then a PE broadcast and a
DVE multiply.  None of the drain work touches the score-tile PSUM ring,
so the PE never idles long enough for HAM to re-throttle the clock.

PSUM budget (8 banks): scoresA 2 + scoresB 2 + ctx 2 + denom 1 + bcast 1.

Biases are folded in via an appended ones-row on X^T (contraction 769).
Host does input transposes (free), sharding, and the final [d,q]->[q,d]
untranspose + concat.
"""

import numpy as np

import concourse.bass as bass
import concourse.tile as tile
from concourse import bacc
from concourse import mybir
from concourse.bass_utils import run_bass_kernel_spmd

F32 = mybir.dt.float32
F16 = mybir.dt.float16
DT_MM = F16          # dtype for matmul operands
DT_NP = np.float16   # matching numpy dtype for host-side input prep

BS, SEQ, HID, NH, HD = 4, 2048, 768, 12, 64
NCORES = 8
HPC = 6          # heads per core
FCH = 6          # 128-row chunks of the 768 contraction dim
DSH = HPC * HD   # 384 output features per core


DEBUG = False


def _body(tc, xt_d, wq_d, wk_d, wv_d, mt_d, ot_d, dbg_d=None):
    nc = tc.nc
    Exp = mybir.ActivationFunctionType.Exp

    with tc.tile_pool(name="persist", bufs=1) as persist:
        # Warm the exp table set ASAP (overlaps the input DMAs).
        dummy = persist.tile([1, 1], F32, tag="dummy")
        nc.vector.memset(dummy, 0.0)
        nc.scalar.activation(out=dummy, in_=dummy, func=Exp)

        mtile = persist.tile([128, 16], DT_MM, tag="mtile")
        nc.sync.dma_start(out=mtile, in_=mt_d[:, :])
        mtf = persist.tile([128, 16], F32, tag="mtf")
        nc.vector.tensor_copy(out=mtf, in_=mtile)
        qt = [persist.tile([128, SEQ], DT_MM, tag=f"qt{j}", name=f"qt{j}") for j in range(3)]
        kt = [persist.tile([128, SEQ], DT_MM, tag=f"kt{j}", name=f"kt{j}") for j in range(3)]
        vt = persist.tile([128, 16, DSH], DT_MM, tag="vt")
        xt1 = persist.tile([1, SEQ], DT_MM, tag="x6")
        nc.sync.dma_start(out=xt1, in_=xt_d[768:769, :])

        # ---------------- Phase 1: QKV projections ----------------
        with tc.tile_pool(name="xw", bufs=1) as xw, \
             tc.tile_pool(name="qkp", bufs=3, space="PSUM") as qkp, \
             tc.tile_pool(name="vp", bufs=2, space="PSUM") as vp:
            xts = []
            for f in range(FCH):
                t = xw.tile([128, SEQ], DT_MM, tag=f"x{f}")
                nc.sync.dma_start(out=t, in_=xt_d[f * 128:(f + 1) * 128, :])
                xts.append(t)

            wmap = {}
            for dram, nm in ((wq_d, "q"), (wk_d, "k"), (wv_d, "v")):
                lst = []
                for f in range(FCH):
                    t = xw.tile([128, DSH], DT_MM, tag=f"w{nm}{f}")
                    nc.sync.dma_start(out=t, in_=dram[f * 128:(f + 1) * 128, :])
                    lst.append(t)
                b = xw.tile([1, DSH], DT_MM, tag=f"w{nm}b")
                nc.sync.dma_start(out=b, in_=dram[768:769, :])
                lst.append(b)
                wmap[nm] = lst

            # Q^T, K^T: [384, 2048] as 3 tiles of [128, 2048]
            for nm, dst in (("q", qt), ("k", kt)):
                wt = wmap[nm]
                for j in range(3):
                    js = slice(j * 128, (j + 1) * 128)
                    for qc in range(4):
                        qs = slice(qc * 512, (qc + 1) * 512)
                        ps = qkp.tile([128, 512], F32, tag="qk")
                        for f in range(FCH):
                            nc.tensor.matmul(ps, lhsT=wt[f][:, js],
                                             rhs=xts[f][:, qs],
                                             start=(f == 0), stop=False)
                        nc.tensor.matmul(ps, lhsT=wt[6][:, js],
                                         rhs=xt1[:, qs],
                                         start=False, stop=True)
                        nc.scalar.copy(out=dst[j][:, qs], in_=ps)

            # V: [2048, 384] as 16 k-blocks, mask-scaled rows
            wt = wmap["v"]
            for kb in range(16):
                ks = slice(kb * 128, (kb + 1) * 128)
                ps = vp.tile([128, DSH], F32, tag="v")
                for f in range(FCH):
                    nc.tensor.matmul(ps, lhsT=xts[f][:, ks], rhs=wt[f],
                                     start=(f == 0), stop=False)
                nc.tensor.matmul(ps, lhsT=xt1[:, ks], rhs=wt[6],
                                 start=False, stop=True)
                nc.vector.tensor_scalar_mul(
                    out=vt[:, kb, :], in0=ps,
                    scalar1=mtf[:, kb:kb + 1])

        # ---------------- Phase 2: attention ----------------
        # PSUM banks: scoresA 2 + scoresB 2 + ctx 2 + denom 1 + bcast 1 = 8
        with tc.tile_pool(name="spA", bufs=1, space="PSUM") as spA, \
             tc.tile_pool(name="spB", bufs=1, space="PSUM") as spB, \
             tc.tile_pool(name="cp", bufs=1, space="PSUM") as cp, \
             tc.tile_pool(name="dp", bufs=1, space="PSUM") as dp, \
             tc.tile_pool(name="bp", bufs=1, space="PSUM") as bp, \
             tc.tile_pool(name="pp", bufs=2) as pp, \
             tc.tile_pool(name="ctp", bufs=2) as ctp, \
             tc.tile_pool(name="rdp", bufs=2) as rdp, \
             tc.tile_pool(name="osp", bufs=3) as osp:
            for j in range(3):
                h0, h1 = 2 * j, 2 * j + 1
                ostage = {h: osp.tile([64, SEQ], F32, tag="os", name=f"os{h}")
                          for h in (h0, h1)}
                for qh in range(2):
                    q0 = qh * 1024
                    cab = cp.tile([128, 1024], F32, tag="c", name="cab")
                    dnt = dp.tile([97, 512], F32, tag="d", name="dnt")
                    for kb in range(16):
                        ks = slice(kb * 128, (kb + 1) * 128)
                        sA = spA.tile([128, 1024], F32, tag="sA", name="sA")
                        sB = spB.tile([128, 1024], F32, tag="sB", name="sB")
                        # scores: the two heads use disjoint 64-row PE
                        # groups -> concurrent on the array
                        for qq in range(2):
                            qs = slice(q0 + qq * 512, q0 + (qq + 1) * 512)
                            osl = slice(qq * 512, (qq + 1) * 512)
                            nc.tensor.matmul(sA[:, osl],
                                             lhsT=kt[j][0:64, ks],
                                             rhs=qt[j][0:64, qs],
                                             start=True, stop=True)
                            nc.tensor.matmul(sB[:, osl],
                                             lhsT=kt[j][64:128, ks],
                                             rhs=qt[j][64:128, qs],
                                             start=True, stop=True)
                        pA = pp.tile([128, 1024], DT_MM, tag="pA", name="pA")
                        nc.scalar.activation(out=pA, in_=sA, func=Exp,
                                             scale=0.125)
                        pB = pp.tile([128, 1024], DT_MM, tag="pB", name="pB")
                        nc.scalar.activation(out=pB, in_=sB, func=Exp,
                                             scale=0.125)
                        st, sp_ = (kb == 0), (kb == 15)
                        for qq in range(2):
                            osl = slice(qq * 512, (qq + 1) * 512)
                            nc.tensor.matmul(
                                cab[0:64, osl],
                                lhsT=vt[:, kb, h0 * 64:(h0 + 1) * 64],
                                rhs=pA[:, osl], start=st, stop=sp_,
                                skip_group_check=True)
                            nc.tensor.matmul(
                                cab[64:128, osl],
                                lhsT=vt[:, kb, h1 * 64:(h1 + 1) * 64],
                                rhs=pB[:, osl], start=st, stop=sp_,
                                skip_group_check=True)
                        # denominators: 4-way col-tiled m=1 matmuls
                        # rows 0/32 = heads A/B cols 0:512; 64/96 = 512:1024
                        for r, p, qq in ((0, pA, 0), (32, pB, 0),
                                         (64, pA, 1), (96, pB, 1)):
                            osl = slice(qq * 512, (qq + 1) * 512)
                            nc.tensor.matmul(dnt[r:r + 1, :],
                                             lhsT=mtile[:, kb:kb + 1],
                                             rhs=p[:, osl],
                                             start=st, stop=sp_,
                                             tile_position=(0, r),
                                             skip_group_check=True)
                    # ---- drain: out = ctx * (1/denom) ----
                    ct0 = ctp.tile([64, 1024], F32, tag="ct0", name="ct0")
                    nc.vector.tensor_copy(out=ct0, in_=cab[0:64, :])
                    ct1 = ctp.tile([64, 1024], F32, tag="ct1", name="ct1")
                    nc.vector.tensor_copy(out=ct1, in_=cab[64:128, :])
                    # engine partition bases must be 32-aligned: pack the
                    # two denominator rows at partitions 0 and 32
                    rd = rdp.tile([33, 1024], F32, tag="rd", name="rd")
                    nc.vector.tensor_copy(out=rd[0:1, 0:512], in_=dnt[0:1, :])
                    nc.vector.tensor_copy(out=rd[32:33, 0:512], in_=dnt[32:33, :])
                    nc.vector.tensor_copy(out=rd[0:1, 512:1024], in_=dnt[64:65, :])
                    nc.vector.tensor_copy(out=rd[32:33, 512:1024], in_=dnt[96:97, :])
                    # single base-0 call: the custom DVE op mishandles
                    # non-zero base partitions (rows 1..31 are unused junk)
                    rcp = rdp.tile([33, 1024], F32, tag="rcp", name="rcp")
                    nc.vector.reciprocal_approx_fast(out=rcp, in_=rd)
                    # fp16, base-partition-0 copies for the PE broadcast
                    rcpA = rdp.tile([1, 1024], DT_MM, tag="rcpA", name="rcpA")
                    nc.vector.tensor_copy(out=rcpA, in_=rcp[0:1, :])
                    rcpB = rdp.tile([1, 1024], DT_MM, tag="rcpB", name="rcpB")
                    nc.vector.tensor_copy(out=rcpB, in_=rcp[32:33, :])
                    if DEBUG and j == 0 and qh == 0:
                        dbg = persist.tile([128, 4096], F32, tag="dbg")
                        nc.vector.tensor_copy(out=dbg[0:33, 0:1024], in_=rd)
                        nc.vector.tensor_copy(out=dbg[0:33, 1024:2048], in_=rcp)
                        nc.vector.tensor_copy(out=dbg[0:64, 2048:3072], in_=ct0)
                        nc.vector.tensor_copy(out=dbg[64:128, 2048:3072], in_=ct1)
                        nc.vector.tensor_copy(out=dbg[0:97, 3072:3584],
                                              in_=dnt[0:97, :])
                        nc.sync.dma_start(out=dbg_d[:, :], in_=dbg)
                    for qq in range(2):
                        osl = slice(qq * 512, (qq + 1) * 512)
                        bc = bp.tile([128, 512], F32, tag="bc", name="bc")
                        nc.tensor.matmul(bc[0:64, :], lhsT=xt1[:, 0:64],
                                         rhs=rcpA[:, osl],
                                         start=True, stop=True,
                                         tile_position=(0, 0),
                                         skip_group_check=True)
                        nc.tensor.matmul(bc[64:128, :], lhsT=xt1[:, 0:64],
                                         rhs=rcpB[:, osl],
                                         start=True, stop=True,
                                         tile_position=(0, 64),
                                         skip_group_check=True)
                        qsl = slice(q0 + qq * 512, q0 + (qq + 1) * 512)
                        nc.vector.tensor_mul(out=ostage[h0][:, qsl],
                                             in0=ct0[:, osl], in1=bc[0:64, :])
                        nc.vector.tensor_mul(out=ostage[h1][:, qsl],
                                             in0=ct1[:, osl], in1=bc[64:128, :])
                for h in (h0, h1):
                    nc.sync.dma_start(out=ot_d[h], in_=ostage[h])


def build_nc():
    nc = bacc.Bacc("TRN2")
    xt_d = nc.declare_dram_parameter("xt", [HID + 1, SEQ], DT_MM, isOutput=False)
    wq_d = nc.declare_dram_parameter("wqT", [HID + 1, DSH], DT_MM, isOutput=False)
    wk_d = nc.declare_dram_parameter("wkT", [HID + 1, DSH], DT_MM, isOutput=False)
    wv_d = nc.declare_dram_parameter("wvT", [HID + 1, DSH], DT_MM, isOutput=False)
    mt_d = nc.declare_dram_parameter("mt", [128, 16], DT_MM, isOutput=False)
    ot_d = nc.declare_dram_parameter("OT", [HPC, HD, SEQ], F32, isOutput=True)
    dbg_d = None
    if DEBUG:
        dbg_d = nc.declare_dram_parameter("DBG", [128, 4096], F32, isOutput=True)
    with tile.TileContext(nc) as tc:
        _body(tc, xt_d, wq_d, wk_d, wv_d, mt_d, ot_d, dbg_d)
    nc.finalize()
    return nc


_NC_CACHE = None


def _get_nc():
    global _NC_CACHE
    if _NC_CACHE is None:
        _NC_CACHE = build_nc()
    return _NC_CACHE


def make_in_maps(hidden_states, attention_mask, Wq, bq, Wk, bk, Wv, bv):
    in_maps = []
    for c in range(NCORES):
        b, g = c // 2, c % 2
        hs = slice(g * DSH, (g + 1) * DSH)
        xt = np.empty((HID + 1, SEQ), DT_NP)
        xt[:HID] = hidden_states[b].T
        xt[HID] = 1.0
        m = (attention_mask[b, 0, 0] > -1).astype(DT_NP)
        mt = np.ascontiguousarray(m.reshape(16, 128).T)

        def aug(W, bias):
            wa = np.empty((HID + 1, DSH), DT_NP)
            wa[:HID] = W[hs, :].T
            wa[HID] = bias[hs]
            return wa

        in_maps.append({
            "xt": np.ascontiguousarray(xt),
            "wqT": aug(Wq, bq),
            "wkT": aug(Wk, bk),
            "wvT": aug(Wv, bv),
            "mt": mt,
        })
    return in_maps


def gather_out(results):
    out = np.empty((BS, SEQ, HID), np.float32)
    for c in range(NCORES):
        b, g = c // 2, c % 2
        ot = results[c]["OT"]  # [6, 64, 2048]
        out[b, :, g * DSH:(g + 1) * DSH] = (
            ot.transpose(2, 0, 1).reshape(SEQ, DSH)
        )
    return out


def kernel(hidden_states, attention_mask, Wq, bq, Wk, bk, Wv, bv):
    nc = _get_nc()
    in_maps = make_in_maps(hidden_states, attention_mask,
                           Wq, bq, Wk, bk, Wv, bv)
    res = run_bass_kernel_spmd(nc, in_maps, core_ids=list(range(NCORES)))
    return gather_out(res.results)


# revision 9
# speedup vs baseline: 1.9094x; 1.1235x over previous
"""BERT self-attention (BS=4, SEQ=2048, HID=768, NH=12) on 8 NeuronCores.

Sharding: core c -> batch b = c//2, head-group g = c%2 (6 heads each).

v3: software-pipelined single-phase design.
  - Attention runs in (pair j, q-chunk of 512) tiles.  Per k-block:
    scores for both heads land in one [128,1024] PSUM tile (row-tiled,
    concurrent on the PE), one ACT exp instruction covers both heads
    (the ACT engine is the kernel's throughput floor), and the ctx
    matmuls accumulate [65,512] per head where row 64 is the softmax
    denominator (V carries an appended mask column).
  - QKV projections for pair j+1 are emitted interleaved into the
    attention(j) instruction stream, filling the PE idle slots under
    the ACT-bound steady state and keeping the PE busy enough that the
    HAM clock gate stays at full rate.
  - Drain per (j,qc): reciprocal_approx_fast on the two denominator
    rows, PE broadcast to 64 rows, DVE multiply.  Nothing in the drain
    touches the score-tile ring, so the pipeline never stalls.

PSUM budget (8 banks): proj 2 + scores 4 + ctxA/bcast 1 + ctxB 1.
Biases fold in via an appended ones-row on X^T (contraction 769).
Host does input transposes (free), sharding, and the final
[d,q]->[q,d] untranspose + concat.
"""

from collections import deque

import numpy as np

import concourse.bass as bass
import concourse.tile as tile
from concourse import bacc
from concourse import mybir
from concourse.bass_utils import run_bass_kernel_spmd

F32 = mybir.dt.float32
F16 = mybir.dt.float16
DT_MM = F16
DT_NP = np.float16

BS, SEQ, HID, NH, HD = 4, 2048, 768, 12, 64
NCORES = 8
HPC = 6          # heads per core
FCH = 6          # 128-row chunks of the 768 contraction dim
DSH = HPC * HD   # 384 output features per core


def _body(tc, xt_d, wq_d, wk_d, wv_d, mt_d, ot_d):
    nc = tc.nc
    Exp = mybir.ActivationFunctionType.Exp

    with tc.tile_pool(name="persist", bufs=1) as persist, \
         tc.tile_pool(name="pjp", bufs=2, space="PSUM") as pjp, \
         tc.tile_pool(name="sp", bufs=2, space="PSUM") as sp, \
         tc.tile_pool(name="cpA", bufs=1, space="PSUM") as cpA, \
         tc.tile_pool(name="cpB", bufs=1, space="PSUM") as cpB, \
         tc.tile_pool(name="pp", bufs=2) as pp, \
         tc.tile_pool(name="ctp", bufs=2) as ctp, \
         tc.tile_pool(name="rdp", bufs=2) as rdp, \
         tc.tile_pool(name="osp", bufs=3) as osp:
        # Warm the exp table set ASAP (overlaps the input DMAs).
        dummy = persist.tile([1, 1], F32, tag="dummy")
        nc.vector.memset(dummy, 0.0)
        nc.scalar.activation(out=dummy, in_=dummy, func=Exp)

        mtile = persist.tile([128, 16], DT_MM, tag="mtile")
        nc.sync.dma_start(out=mtile, in_=mt_d[:, :])
        mtf = persist.tile([128, 16], F32, tag="mtf")
        nc.vector.tensor_copy(out=mtf, in_=mtile)

        qt = [persist.tile([128, SEQ], DT_MM, tag=f"qt{j}", name=f"qt{j}")
              for j in range(3)]
        kt = [persist.tile([128, SEQ], DT_MM, tag=f"kt{j}", name=f"kt{j}")
              for j in range(3)]
        # per-pair V: [k, 130] = [h0 d 0:64 | mask | h1 d 0:64 | mask]
        vt = [persist.tile([128, 16, 130], DT_MM, tag=f"vt{j}", name=f"vt{j}")
              for j in range(3)]
        xt1 = persist.tile([1, SEQ], DT_MM, tag="x6")
        nc.sync.dma_start(out=xt1, in_=xt_d[768:769, :])

        xts = []
        for f in range(FCH):
            t = persist.tile([128, SEQ], DT_MM, tag=f"x{f}", name=f"x{f}")
            nc.sync.dma_start(out=t, in_=xt_d[f * 128:(f + 1) * 128, :])
            xts.append(t)
        wmap = {}
        for dram, nm in ((wq_d, "q"), (wk_d, "k"), (wv_d, "v")):
            lst = []
            for f in range(FCH):
                t = persist.tile([128, DSH], DT_MM, tag=f"w{nm}{f}",
                                 name=f"w{nm}{f}")
                nc.sync.dma_start(out=t, in_=dram[f * 128:(f + 1) * 128, :])
                lst.append(t)
            b = persist.tile([1, DSH], DT_MM, tag=f"w{nm}b", name=f"w{nm}b")
            nc.sync.dma_start(out=b, in_=dram[768:769, :])
            lst.append(b)
            wmap[nm] = lst

        # mask columns of V (written once per pair)
        for j in range(3):
            nc.vector.tensor_copy(out=vt[j][:, :, 64], in_=mtf)
            nc.vector.tensor_copy(out=vt[j][:, :, 129], in_=mtf)

        # ---- projection chunk emitters (PSUM via the 2-bank pjp ring) ----
        def qk_chunk(nm, j, qc):
            js = slice(j * 128, (j + 1) * 128)
            qs = slice(qc * 512, (qc + 1) * 512)
            wt = wmap[nm]
            ps = pjp.tile([128, 512], F32, tag="pj", name="pj")
            for f in range(FCH):
                nc.tensor.matmul(ps, lhsT=wt[f][:, js], rhs=xts[f][:, qs],
                                 start=(f == 0), stop=False)
            nc.tensor.matmul(ps, lhsT=wt[6][:, js], rhs=xt1[:, qs],
                             start=False, stop=True)
            dst = qt[j] if nm == "q" else kt[j]
            nc.vector.tensor_copy(out=dst[:, qs], in_=ps)

        def v_chunk(j, kb):
            js = slice(j * 128, (j + 1) * 128)
            ks = slice(kb * 128, (kb + 1) * 128)
            wt = wmap["v"]
            ps = pjp.tile([128, 512], F32, tag="pj", name="pj")
            for f in range(FCH):
                nc.tensor.matmul(ps[:, 0:128], lhsT=xts[f][:, ks],
                                 rhs=wt[f][:, js],
                                 start=(f == 0), stop=False)
            nc.tensor.matmul(ps[:, 0:128], lhsT=xt1[:, ks],
                             rhs=wt[6][:, js], start=False, stop=True)
            nc.vector.tensor_scalar_mul(out=vt[j][:, kb, 0:64],
                                        in0=ps[:, 0:64],
                                        scalar1=mtf[:, kb:kb + 1])
            nc.vector.tensor_scalar_mul(out=vt[j][:, kb, 65:129],
                                        in0=ps[:, 64:128],
                                        scalar1=mtf[:, kb:kb + 1])

        def proj_chunks(j):
            out = []
            for kb in range(16):
                out.append(lambda kb=kb: v_chunk(j, kb))
            for nm in ("q", "k"):
                for qc in range(4):
                    out.append(lambda nm=nm, qc=qc: qk_chunk(nm, j, qc))
            return deque(out)

        # lead-in: pair 0 projections
        for em in proj_chunks(0):
            em()

        # ---- attention with pair j+1 projections interleaved ----
        for j in range(3):
            h0, h1 = 2 * j, 2 * j + 1
            pending = proj_chunks(j + 1) if j < 2 else deque()
            ostage = {h: osp.tile([64, SEQ], F32, tag="os", name=f"os{h}")
                      for h in (h0, h1)}
            for qc in range(4):
                qs = slice(qc * 512, (qc + 1) * 512)
                ctxA = cpA.tile([65, 512], F32, tag="cA", name="ctxA")
                ctxB = cpB.tile([65, 512], F32, tag="cB", name="ctxB")
                for kb in range(16):
                    ks = slice(kb * 128, (kb + 1) * 128)
                    sab = sp.tile([128, 1024], F32, tag="s", name="sab")
                    nc.tensor.matmul(sab[:, 0:512],
                                     lhsT=kt[j][0:64, ks],
                                     rhs=qt[j][0:64, qs],
                                     start=True, stop=True)
                    nc.tensor.matmul(sab[:, 512:1024],
                                     lhsT=kt[j][64:128, ks],
                                     rhs=qt[j][64:128, qs],
                                     start=True, stop=True)
                    pab = pp.tile([128, 1024], DT_MM, tag="p", name="pab")
                    nc.scalar.activation(out=pab, in_=sab, func=Exp,
                                         scale=0.125)
                    st, sp_ = (kb == 0), (kb == 15)
                    nc.tensor.matmul(ctxA, lhsT=vt[j][:, kb, 0:65],
                                     rhs=pab[:, 0:512],
                                     start=st, stop=sp_,
                                     skip_group_check=True)
                    nc.tensor.matmul(ctxB, lhsT=vt[j][:, kb, 65:130],
                                     rhs=pab[:, 512:1024],
                                     start=st, stop=sp_,
                                     skip_group_check=True)
                    if pending and kb % 2 == 1:
                        pending.popleft()()
                # ---- drain ----
                ct = ctp.tile([128, 512], F32, tag="ct", name="ct")
                nc.vector.tensor_copy(out=ct[0:64, :], in_=ctxA[0:64, :])
                nc.vector.tensor_copy(out=ct[64:128, :], in_=ctxB[0:64, :])
                rd = rdp.tile([33, 512], F32, tag="rd", name="rd")
                nc.vector.tensor_copy(out=rd[0:1, :], in_=ctxA[64:65, :])
                nc.vector.tensor_copy(out=rd[32:33, :], in_=ctxB[64:65, :])
                rcp = rdp.tile([33, 512], F32, tag="rcp", name="rcp")
                nc.vector.reciprocal_approx_fast(out=rcp, in_=rd)
                rcpA = rdp.tile([1, 512], DT_MM, tag="rcpA", name="rcpA")
                nc.vector.tensor_copy(out=rcpA, in_=rcp[0:1, :])
                rcpB = rdp.tile([1, 512], DT_MM, tag="rcpB", name="rcpB")
                nc.vector.tensor_copy(out=rcpB, in_=rcp[32:33, :])
                bc = cpA.tile([128, 512], F32, tag="cA", name="bc")
                nc.tensor.matmul(bc[0:64, :], lhsT=xt1[:, 0:64], rhs=rcpA,
                                 start=True, stop=True,
                                 tile_position=(0, 0),
                                 skip_group_check=True)
                nc.tensor.matmul(bc[64:128, :], lhsT=xt1[:, 0:64], rhs=rcpB,
                                 start=True, stop=True,
                                 tile_position=(0, 64),
                                 skip_group_check=True)
                nc.vector.tensor_mul(out=ostage[h0][:, qs],
                                     in0=ct[0:64, :], in1=bc[0:64, :])
                nc.vector.tensor_mul(out=ostage[h1][:, qs],
                                     in0=ct[64:128, :], in1=bc[64:128, :])
            while pending:
                pending.popleft()()
            for h in (h0, h1):
                nc.sync.dma_start(out=ot_d[h], in_=ostage[h])


def build_nc():
    nc = bacc.Bacc("TRN2")
    xt_d = nc.declare_dram_parameter("xt", [HID + 1, SEQ], DT_MM, isOutput=False)
    wq_d = nc.declare_dram_parameter("wqT", [HID + 1, DSH], DT_MM, isOutput=False)
    wk_d = nc.declare_dram_parameter("wkT", [HID + 1, DSH], DT_MM, isOutput=False)
    wv_d = nc.declare_dram_parameter("wvT", [HID + 1, DSH], DT_MM, isOutput=False)
    mt_d = nc.declare_dram_parameter("mt", [128, 16], DT_MM, isOutput=False)
    ot_d = nc.declare_dram_parameter("OT", [HPC, HD, SEQ], F32, isOutput=True)
    with tile.TileContext(nc) as tc:
        _body(tc, xt_d, wq_d, wk_d, wv_d, mt_d, ot_d)
    nc.finalize()
    return nc


_NC_CACHE = None


def _get_nc():
    global _NC_CACHE
    if _NC_CACHE is None:
        _NC_CACHE = build_nc()
    return _NC_CACHE


def make_in_maps(hidden_states, attention_mask, Wq, bq, Wk, bk, Wv, bv):
    in_maps = []
    for c in range(NCORES):
        b, g = c // 2, c % 2
        hs = slice(g * DSH, (g + 1) * DSH)
        xt = np.empty((HID + 1, SEQ), DT_NP)
        xt[:HID] = hidden_states[b].T
        xt[HID] = 1.0
        m = (attention_mask[b, 0, 0] > -1).astype(DT_NP)
        mt = np.ascontiguousarray(m.reshape(16, 128).T)

        def aug(W, bias):
            wa = np.empty((HID + 1, DSH), DT_NP)
            wa[:HID] = W[hs, :].T
            wa[HID] = bias[hs]
            return wa

        in_maps.append({
            "xt": np.ascontiguousarray(xt),
            "wqT": aug(Wq, bq),
            "wkT": aug(Wk, bk),
            "wvT": aug(Wv, bv),
            "mt": mt,
        })
    return in_maps


def gather_out(results):
    out = np.empty((BS, SEQ, HID), np.float32)
    for c in range(NCORES):
        b, g = c // 2, c % 2
        ot = results[c]["OT"]  # [6, 64, 2048]
        out[b, :, g * DSH:(g + 1) * DSH] = (
            ot.transpose(2, 0, 1).reshape(SEQ, DSH)
        )
    return out


def kernel(hidden_states, attention_mask, Wq, bq, Wk, bk, Wv, bv):
    nc = _get_nc()
    in_maps = make_in_maps(hidden_states, attention_mask,
                           Wq, bq, Wk, bk, Wv, bv)
    res = run_bass_kernel_spmd(nc, in_maps, core_ids=list(range(NCORES)))
    return gather_out(res.results)


# revision 13
# speedup vs baseline: 2.1611x; 1.1318x over previous
"""BERT self-attention (BS=4, SEQ=2048, HID=768, NH=12) on 8 NeuronCores.

Sharding: core c -> batch b = c//2, head-group g = c%2 (6 heads each).

v3: software-pipelined single-phase design.
  - Attention runs in (pair j, q-chunk of 512) tiles.  Per k-block:
    scores for both heads land in one [128,1024] PSUM tile (row-tiled,
    concurrent on the PE), one ACT exp instruction covers both heads
    (the ACT engine is the kernel's throughput floor), and the ctx
    matmuls accumulate [65,512] per head where row 64 is the softmax
    denominator (V carries an appended mask column).
  - QKV projections for pair j+1 are emitted interleaved into the
    attention(j) instruction stream, filling the PE idle slots under
    the ACT-bound steady state and keeping the PE busy enough that the
    HAM clock gate stays at full rate.
  - Drain per (j,qc): reciprocal_approx_fast on the two denominator
    rows, PE broadcast to 64 rows, DVE multiply.  Nothing in the drain
    touches the score-tile ring, so the pipeline never stalls.

PSUM budget (8 banks): proj 2 + scores 4 + ctxA/bcast 1 + ctxB 1.
Biases fold in via an appended ones-row on X^T (contraction 769).
Host does input transposes (free), sharding, and the final
[d,q]->[q,d] untranspose + concat.
"""

from collections import deque

import numpy as np

import concourse.bass as bass
import concourse.tile as tile
from concourse import bacc
from concourse import mybir
from concourse.bass_utils import run_bass_kernel_spmd

F32 = mybir.dt.float32
F16 = mybir.dt.float16
DT_MM = F16
DT_NP = np.float16

BS, SEQ, HID, NH, HD = 4, 2048, 768, 12, 64
NCORES = 8
HPC = 6          # heads per core
FCH = 6          # 128-row chunks of the 768 contraction dim
DSH = HPC * HD   # 384 output features per core


def _body(tc, xt_d, wq_d, wk_d, wv_d, mt_d, ot_d):
    nc = tc.nc
    Exp = mybir.ActivationFunctionType.Exp

    with tc.tile_pool(name="persist", bufs=1) as persist, \
         tc.tile_pool(name="pjp", bufs=2, space="PSUM") as pjp, \
         tc.tile_pool(name="sp", bufs=2, space="PSUM") as sp, \
         tc.tile_pool(name="cpA", bufs=1, space="PSUM") as cpA, \
         tc.tile_pool(name="cpB", bufs=1, space="PSUM") as cpB, \
         tc.tile_pool(name="pp", bufs=2) as pp, \
         tc.tile_pool(name="ctp", bufs=2) as ctp, \
         tc.tile_pool(name="rdp", bufs=2) as rdp, \
         tc.tile_pool(name="osp", bufs=3) as osp:
        # Warm the exp table set ASAP (overlaps the input DMAs).
        dummy = persist.tile([1, 1], F32, tag="dummy")
        nc.vector.memset(dummy, 0.0)
        nc.scalar.activation(out=dummy, in_=dummy, func=Exp)

        mtile = persist.tile([128, 16], DT_MM, tag="mtile")
        nc.sync.dma_start(out=mtile, in_=mt_d[:, :])
        mtf = persist.tile([128, 16], F32, tag="mtf")
        nc.vector.tensor_copy(out=mtf, in_=mtile)

        qt = [persist.tile([128, SEQ], DT_MM, tag=f"qt{j}", name=f"qt{j}")
              for j in range(3)]
        kt = [persist.tile([128, SEQ], DT_MM, tag=f"kt{j}", name=f"kt{j}")
              for j in range(3)]
        # per-pair V: [k, 130] = [h0 d 0:64 | mask | h1 d 0:64 | mask]
        vt = [persist.tile([128, 16, 130], DT_MM, tag=f"vt{j}", name=f"vt{j}")
              for j in range(3)]
        xt1 = persist.tile([1, SEQ], DT_MM, tag="x6")
        nc.sync.dma_start(out=xt1, in_=xt_d[768:769, :])

        # interleave W and X DMAs so the first projection chunk can start
        # as soon as the first (w, x) tile pair lands
        xts = []
        wmap = {"q": [], "k": [], "v": []}
        wdram = {"q": wq_d, "k": wk_d, "v": wv_d}
        for f in range(FCH):
            t = persist.tile([128, DSH], DT_MM, tag=f"wq{f}", name=f"wq{f}")
            nc.sync.dma_start(out=t, in_=wq_d[f * 128:(f + 1) * 128, :])
            wmap["q"].append(t)
            t = persist.tile([128, SEQ], DT_MM, tag=f"x{f}", name=f"x{f}")
            nc.sync.dma_start(out=t, in_=xt_d[f * 128:(f + 1) * 128, :])
            xts.append(t)
        b = persist.tile([1, DSH], DT_MM, tag="wqb", name="wqb")
        nc.sync.dma_start(out=b, in_=wq_d[768:769, :])
        wmap["q"].append(b)
        for nm in ("k", "v"):
            for f in range(FCH):
                t = persist.tile([128, DSH], DT_MM, tag=f"w{nm}{f}",
                                 name=f"w{nm}{f}")
                nc.sync.dma_start(out=t, in_=wdram[nm][f * 128:(f + 1) * 128, :])
                wmap[nm].append(t)
            b = persist.tile([1, DSH], DT_MM, tag=f"w{nm}b", name=f"w{nm}b")
            nc.sync.dma_start(out=b, in_=wdram[nm][768:769, :])
            wmap[nm].append(b)

        # mask columns of V (written once per pair)
        for j in range(3):
            nc.vector.tensor_copy(out=vt[j][:, :, 64], in_=mtf)
            nc.vector.tensor_copy(out=vt[j][:, :, 129], in_=mtf)

        # ---- projection chunk emitters (PSUM via the 2-bank pjp ring) ----
        def qk_chunk(nm, j, qc):
            js = slice(j * 128, (j + 1) * 128)
            qs = slice(qc * 512, (qc + 1) * 512)
            wt = wmap[nm]
            ps = pjp.tile([128, 512], F32, tag="pj", name="pj")
            for f in range(FCH):
                nc.tensor.matmul(ps, lhsT=wt[f][:, js], rhs=xts[f][:, qs],
                                 start=(f == 0), stop=False)
            nc.tensor.matmul(ps, lhsT=wt[6][:, js], rhs=xt1[:, qs],
                             start=False, stop=True)
            dst = qt[j] if nm == "q" else kt[j]
            nc.vector.tensor_copy(out=dst[:, qs], in_=ps)

        def v_chunk(j, kb):
            js = slice(j * 128, (j + 1) * 128)
            ks = slice(kb * 128, (kb + 1) * 128)
            wt = wmap["v"]
            ps = pjp.tile([128, 512], F32, tag="pj", name="pj")
            for f in range(FCH):
                nc.tensor.matmul(ps[:, 0:128], lhsT=xts[f][:, ks],
                                 rhs=wt[f][:, js],
                                 start=(f == 0), stop=False)
            nc.tensor.matmul(ps[:, 0:128], lhsT=xt1[:, ks],
                             rhs=wt[6][:, js], start=False, stop=True)
            nc.vector.tensor_scalar_mul(out=vt[j][:, kb, 0:64],
                                        in0=ps[:, 0:64],
                                        scalar1=mtf[:, kb:kb + 1])
            nc.vector.tensor_scalar_mul(out=vt[j][:, kb, 65:129],
                                        in0=ps[:, 64:128],
                                        scalar1=mtf[:, kb:kb + 1])

        def proj_chunks(j):
            out = []
            for kb in range(16):
                out.append(lambda kb=kb: v_chunk(j, kb))
            for nm in ("q", "k"):
                for qc in range(4):
                    out.append(lambda nm=nm, qc=qc: qk_chunk(nm, j, qc))
            return deque(out)

        # lead-in: pair-0 Q for qc0 plus ALL of pair-0 K (scores at any qc
        # read the full key sequence); remaining Q chunks interleave
        qk_chunk("q", 0, 0)
        for qc in range(4):
            qk_chunk("k", 0, qc)
        pending = deque()
        for qc in range(1, 4):
            pending.append(lambda qc=qc: qk_chunk("q", 0, qc))

        # drain finisher (bc broadcast + multiplies), delayed into the next
        # q-chunk's kb loop so the PE FIFO never stalls on the DVE chain
        finisher = [None]

        def run_finisher():
            if finisher[0] is not None:
                finisher[0]()
                finisher[0] = None

        for j in range(3):
            h0, h1 = 2 * j, 2 * j + 1
            if j < 2:
                pending.extend(proj_chunks(j + 1))
            ostage = {h: osp.tile([64, SEQ], F32, tag="os", name=f"os{h}")
                      for h in (h0, h1)}
            for qc in range(4):
                qs = slice(qc * 512, (qc + 1) * 512)
                ctxA = cpA.tile([65, 512], F32, tag="cA", name="ctxA")
                ctxB = cpB.tile([65, 512], F32, tag="cB", name="ctxB")
                pabs = [None, None]   # pab of kb-1, kb
                first_j0 = (j == 0 and qc == 0)
                for kb in range(16):
                    ks = slice(kb * 128, (kb + 1) * 128)
                    if first_j0:
                        # V chunks of pair 0, just in time for ctx(kb)
                        v_chunk(0, kb)
                    sab = sp.tile([128, 1024], F32, tag="s", name="sab")
                    nc.tensor.matmul(sab[:, 0:512],
                                     lhsT=kt[j][0:64, ks],
                                     rhs=qt[j][0:64, qs],
                                     start=True, stop=True)
                    nc.tensor.matmul(sab[:, 512:1024],
                                     lhsT=kt[j][64:128, ks],
                                     rhs=qt[j][64:128, qs],
                                     start=True, stop=True)
                    pab = pp.tile([128, 1024], DT_MM, tag="p", name="pab")
                    nc.scalar.activation(out=pab, in_=sab, func=Exp,
                                         scale=0.125)
                    pabs[1] = pab
                    if kb == 2:
                        run_finisher()
                    # ctx one step behind scores: the PE FIFO never waits
                    # on the exp that was just queued
                    if kb > 0:
                        st, sp_ = (kb == 1), False
                        pprev = pabs[0]
                        nc.tensor.matmul(ctxA, lhsT=vt[j][:, kb - 1, 0:65],
                                         rhs=pprev[:, 0:512],
                                         start=st, stop=sp_,
                                         skip_group_check=True)
                        nc.tensor.matmul(ctxB, lhsT=vt[j][:, kb - 1, 65:130],
                                         rhs=pprev[:, 512:1024],
                                         start=st, stop=sp_,
                                         skip_group_check=True)
                    pabs[0] = pab
                    if pending and (kb % 4 == 3 if first_j0 else kb % 2 == 1):
                        pending.popleft()()
                nc.tensor.matmul(ctxA, lhsT=vt[j][:, 15, 0:65],
                                 rhs=pabs[0][:, 0:512],
                                 start=False, stop=True,
                                 skip_group_check=True)
                nc.tensor.matmul(ctxB, lhsT=vt[j][:, 15, 65:130],
                                 rhs=pabs[0][:, 512:1024],
                                 start=False, stop=True,
                                 skip_group_check=True)
                # ---- drain part 1: DVE only ----
                ct = ctp.tile([128, 512], F32, tag="ct", name="ct")
                nc.vector.tensor_copy(out=ct[0:64, :], in_=ctxA[0:64, :])
                nc.vector.tensor_copy(out=ct[64:128, :], in_=ctxB[0:64, :])
                rd = rdp.tile([33, 512], F32, tag="rd", name="rd")
                nc.vector.tensor_copy(out=rd[0:1, :], in_=ctxA[64:65, :])
                nc.vector.tensor_copy(out=rd[32:33, :], in_=ctxB[64:65, :])
                rcp = rdp.tile([33, 512], F32, tag="rcp", name="rcp")
                nc.vector.reciprocal_approx_fast(out=rcp, in_=rd)
                rcpA = rdp.tile([1, 512], DT_MM, tag="rcpA", name="rcpA")
                nc.vector.tensor_copy(out=rcpA, in_=rcp[0:1, :])
                rcpB = rdp.tile([1, 512], DT_MM, tag="rcpB", name="rcpB")
                nc.vector.tensor_copy(out=rcpB, in_=rcp[32:33, :])

                def fin(ct=ct, rcpA=rcpA, rcpB=rcpB, qs=qs,
                        osA=ostage[h0], osB=ostage[h1]):
                    bc = pjp.tile([128, 512], F32, tag="pj", name="bc")
                    nc.tensor.matmul(bc[0:64, :], lhsT=xt1[:, 0:64],
                                     rhs=rcpA, start=True, stop=True,
                                     tile_position=(0, 0),
                                     skip_group_check=True)
                    nc.tensor.matmul(bc[64:128, :], lhsT=xt1[:, 0:64],
                                     rhs=rcpB, start=True, stop=True,
                                     tile_position=(0, 64),
                                     skip_group_check=True)
                    nc.vector.tensor_mul(out=osA[:, qs],
                                         in0=ct[0:64, :], in1=bc[0:64, :])
                    nc.vector.tensor_mul(out=osB[:, qs],
                                         in0=ct[64:128, :], in1=bc[64:128, :])
                finisher[0] = fin
            while pending:
                pending.popleft()()
            # qc3's finisher must precede the ostage DMA emission (the DMA
            # only orders against prior writers in program order)
            run_finisher()
            for h in (h0, h1):
                nc.sync.dma_start(out=ot_d[h], in_=ostage[h])


def build_nc():
    nc = bacc.Bacc("TRN2")
    xt_d = nc.declare_dram_parameter("xt", [HID + 1, SEQ], DT_MM, isOutput=False)
    wq_d = nc.declare_dram_parameter("wqT", [HID + 1, DSH], DT_MM, isOutput=False)
    wk_d = nc.declare_dram_parameter("wkT", [HID + 1, DSH], DT_MM, isOutput=False)
    wv_d = nc.declare_dram_parameter("wvT", [HID + 1, DSH], DT_MM, isOutput=False)
    mt_d = nc.declare_dram_parameter("mt", [128, 16], DT_MM, isOutput=False)
    ot_d = nc.declare_dram_parameter("OT", [HPC, HD, SEQ], F32, isOutput=True)
    with tile.TileContext(nc) as tc:
        _body(tc, xt_d, wq_d, wk_d, wv_d, mt_d, ot_d)
    nc.finalize()
    return nc


_NC_CACHE = None


def _get_nc():
    global _NC_CACHE
    if _NC_CACHE is None:
        _NC_CACHE = build_nc()
    return _NC_CACHE


def make_in_maps(hidden_states, attention_mask, Wq, bq, Wk, bk, Wv, bv):
    in_maps = []
    for c in range(NCORES):
        b, g = c // 2, c % 2
        hs = slice(g * DSH, (g + 1) * DSH)
        xt = np.empty((HID + 1, SEQ), DT_NP)
        xt[:HID] = hidden_states[b].T
        xt[HID] = 1.0
        m = (attention_mask[b, 0, 0] > -1).astype(DT_NP)
        mt = np.ascontiguousarray(m.reshape(16, 128).T)

        def aug(W, bias):
            wa = np.empty((HID + 1, DSH), DT_NP)
            wa[:HID] = W[hs, :].T
            wa[HID] = bias[hs]
            return wa

        in_maps.append({
            "xt": np.ascontiguousarray(xt),
            "wqT": aug(Wq, bq),
            "wkT": aug(Wk, bk),
            "wvT": aug(Wv, bv),
            "mt": mt,
        })
    return in_maps


def gather_out(results):
    out = np.empty((BS, SEQ, HID), np.float32)
    for c in range(NCORES):
        b, g = c // 2, c % 2
        ot = results[c]["OT"]  # [6, 64, 2048]
        out[b, :, g * DSH:(g + 1) * DSH] = (
            ot.transpose(2, 0, 1).reshape(SEQ, DSH)
        )
    return out


def kernel(hidden_states, attention_mask, Wq, bq, Wk, bk, Wv, bv):
    nc = _get_nc()
    in_maps = make_in_maps(hidden_states, attention_mask,
                           Wq, bq, Wk, bk, Wv, bv)
    res = run_bass_kernel_spmd(nc, in_maps, core_ids=list(range(NCORES)))
    return gather_out(res.results)
